# revision 84
# baseline (speedup 1.0000x reference)
"""Trainium2 Bass kernel for causal MultiHeadAttention + residual + LayerNorm.

Problem: nn_MultiHeadAttention_88124138979383
  B=2, L=2048, D=1024, H=16, DH=64, causal mask, out-proj, residual, LN.

Sharding (8 cores): core c = (batch b=c//4, head-group g=c%4, 4 heads each).
Each core projects Q^T/K^T (head-dim on partitions) and V (natural layout)
for its batch+heads, runs causal attention in scores-transposed layout
(softmax reduction via a ones-column appended to V inside the P@V matmul,
no max-subtraction — scores are small), normalizes via a PE-broadcast of
the reciprocal denominator, then per q-block an 8-core AllToAll exchanges
head-shards for sequence-shards: after A2A #qb, core c has the full
16-head A^T for rows [512qb+64c, +64) of BOTH batches; it immediately runs
the output projection (both batches packed on 128 partitions), residual
(query rows + bo pre-added on host) and LayerNorm for those rows, so the
epilogue of q-block qb overlaps attention of q-block qb+1.

All matmuls in bf16 (fp32 PSUM accumulate). Host transposes/casts inputs.
"""
import os
import sys

for _p in ("/opt/trn_rl_repo", os.path.join(os.path.expanduser("~"), ".axon_site", "_ro", "trn_rl_repo")):
    if os.path.isdir(_p) and _p not in sys.path:
        sys.path.insert(0, _p)

import numpy as np
import ml_dtypes

import concourse.bass as bass
import concourse.tile as tile
from concourse import bacc, mybir

BF16 = ml_dtypes.bfloat16
F32 = mybir.dt.float32
BF = mybir.dt.bfloat16

B, L, D = 2, 2048, 1024
H, DH = 16, 64
NCORES = 8
HL = 4                 # heads per core
NPAIR = 2              # head pairs per core
MBS = 512              # m-block size for projections
NMB = L // MBS         # 4
QBS = 512              # q-block size for attention
NQB = L // QBS         # 4
KTS = 128              # k-tile size
NKT = L // KTS         # 16
MS = L // NCORES       # 256: per-core row-slice per A2A block
LN_EPS = 1e-5
SCALE = 1.0 / 8.0      # 1/sqrt(DH)
# q-blocks whose A2A exchanges are merged into one collective (fewer syncs;
# each collective carries ~15us fixed cost)
A2A_GROUPS = [[0, 1], [2, 3]]


def build_nc(reps: int = 1, phases: str = 'full', with_biases: bool = False,
             ln_affine: bool = False):
    nc = bacc.Bacc("TRN2", target_bir_lowering=False, debug=False, num_devices=NCORES)
    qT = nc.dram_tensor("qT", [D, L], BF, kind="ExternalInput")
    kT = nc.dram_tensor("kT", [D, L], BF, kind="ExternalInput")
    vT = nc.dram_tensor("vT", [D, L], BF, kind="ExternalInput")
    wq = nc.dram_tensor("wq", [D, HL * DH], BF, kind="ExternalInput")
    wk = nc.dram_tensor("wk", [D, HL * DH], BF, kind="ExternalInput")
    wv = nc.dram_tensor("wv", [D, HL * DH], BF, kind="ExternalInput")
    wo = nc.dram_tensor("wo", [D, D], BF, kind="ExternalInput")
    bq = nc.dram_tensor("bq", [HL * DH, 1], F32, kind="ExternalInput")
    bk = nc.dram_tensor("bk", [HL * DH, 1], F32, kind="ExternalInput")
    bv = nc.dram_tensor("bv", [DH, HL], F32, kind="ExternalInput")
    # rows ordered (qb, batch, i): row 128*qb + 64*b + i = query[b, 512qb+64c+i] + bo
    qresbo = nc.dram_tensor("qresbo", [4 * 128, D], F32, kind="ExternalInput")
    gamma = nc.dram_tensor("gamma", [1, D], F32, kind="ExternalInput")
    beta = nc.dram_tensor("beta", [1, D], F32, kind="ExternalInput")
    y = nc.dram_tensor("y", [4 * 128, D], F32, kind="ExternalOutput")

    with tile.TileContext(nc) as tc:
        with (
            tc.tile_pool(name="consts", bufs=1) as consts,
            tc.tile_pool(name="persist", bufs=1) as persist,
            tc.tile_pool(name="xin", bufs=2) as xin,
            tc.tile_pool(name="es", bufs=8) as espool,
            tc.tile_pool(name="work", bufs=2) as work,
            tc.tile_pool(name="epi", bufs=1) as epi,
            tc.tile_pool(name="ps_proj", bufs=2, space="PSUM") as ps_proj,
            tc.tile_pool(name="ps_s", bufs=3, space="PSUM") as ps_s,
            tc.tile_pool(name="ps_acc", bufs=2, space="PSUM") as ps_acc,
            tc.tile_pool(name="ps_bc", bufs=1, space="PSUM") as ps_bc,
            tc.tile_pool(name="dram", bufs=1, space="DRAM") as dram,
        ):
            # ---- constants / weights ----
            wq_sb = consts.tile([128, 8, HL * DH], BF, tag="wq")
            wk_sb = consts.tile([128, 8, HL * DH], BF, tag="wk")
            wv_sb = consts.tile([128, 8, HL * DH], BF, tag="wv")

            def load_wkv():
                nc.sync.dma_start(out=wk_sb, in_=wk.rearrange("(t p) n -> p t n", p=128))
                nc.sync.dma_start(out=wv_sb, in_=wv.rearrange("(t p) n -> p t n", p=128))
            wo_sb = consts.tile([128, 8, D], BF, tag="wo")
            bq_sb = consts.tile([128, NPAIR], F32, tag="bq")
            bk_sb = consts.tile([128, NPAIR], F32, tag="bk")
            if with_biases:
                nc.sync.dma_start(out=bq_sb, in_=bq.rearrange("(t p) o -> p (t o)", p=128))
                nc.sync.dma_start(out=bk_sb, in_=bk.rearrange("(t p) o -> p (t o)", p=128))
            bv_sb = consts.tile([DH, HL], F32, tag="bv")
            if with_biases:
                nc.sync.dma_start(out=bv_sb, in_=bv[:, :])
            ones_sb = consts.tile([128, DH], BF, tag="ones")
            nc.gpsimd.memset(ones_sb, 1.0)
            # causal masks for the 4 diagonal-band offsets d = kt - 4*qb:
            # mask_d[p, q] = 1.0 where q >= p + 128*d else 0.0
            cmask_sb = consts.tile([128, 4, QBS], BF, tag="cmask")
            nc.gpsimd.memset(cmask_sb, 1.0)
            for d in range(4):
                nc.gpsimd.affine_select(
                    out=cmask_sb[:, d, :], in_=cmask_sb[:, d, :],
                    compare_op=mybir.AluOpType.is_ge, fill=0.0,
                    base=-128 * d, channel_multiplier=-1, pattern=[[1, QBS]])

            # ---- persistent activations ----
            qT_sb = [persist.tile([128, L], BF, tag=f"qT{p}", name=f"qT_sb{p}") for p in range(NPAIR)]
            kT_sb = [persist.tile([128, L], BF, tag=f"kT{p}", name=f"kT_sb{p}") for p in range(NPAIR)]
            # V in natural [seq, d] layout, 65 cols per head (col 64 = ones)
            v65_sb = persist.tile([128, NKT, HL * 65], BF, tag="v65")
            nc.gpsimd.memset(v65_sb, 1.0)
            # normalized attention output A^T, one tile per head so readers
            # only wait on their own head's writes: [DH, L]
            a4_sb = [persist.tile([DH, L], BF, tag=f"a4_{h}", name=f"a4_sb{h}")
                     for h in range(HL)]
            # gathered A^T after chunked A2As, one tile per q-block so the
            # epilogue of qb only waits on its own gather DMAs. Layout
            # [p, ct, x(batch), m] keeps the out-proj lhsT slice (x, m)
            # contiguous -- walrus requires a single mergeable free dim.
            ob_sb = [persist.tile([128, 8, 2, DH], BF, tag=f"ob{q}", name=f"ob_sb{q}")
                     for q in range(NQB)]
            ob_v = ob_sb

            gam_sb = consts.tile([128, D], F32, tag="gam") if ln_affine else None
            bet_sb = consts.tile([128, D], F32, tag="bet") if ln_affine else None
            qres_sb4 = persist.tile([128, 4, D], F32, tag="qres4", name="qres_sb4")

            # one staging pair per collective GROUP: a group of k q-blocks
            # exchanges [k*2048, 64]; A2A block j' (256k rows, peer j') holds
            # rows j'*256k + qi*256 + 64h + p for each member qi
            in_g = [dram.tile([len(g) * L, DH], BF, name=f"in_g{i}")
                    for i, g in enumerate(A2A_GROUPS)]
            out_g = [dram.tile([len(g) * L, DH], BF, name=f"out_g{i}")
                     for i, g in enumerate(A2A_GROUPS)]
            qb2group = {}
            for gi, g in enumerate(A2A_GROUPS):
                for qi, qb in enumerate(g):
                    qb2group[qb] = (gi, qi)

            def proj_block(mb, split_in=False, qk_copies_on_act=False, parts='all'):
                m0 = mb * MBS
                if parts == 'q':
                    # Q-only part, hoisted one attention block early: the exp
                    # stream of block mb is gated only by qT(mb) (its K/V are
                    # needed just for the last 4 diagonal k-tiles), so copying
                    # Q during attn(mb-1) lets ACT start block mb's exps
                    # without a boundary gap.
                    xq = xin.tile([128, 8, MBS], BF, tag="xq")
                    nc.sync.dma_start(out=xq, in_=qT[:, m0:m0 + MBS].rearrange("(t p) m -> p t m", p=128))
                    for p in range(NPAIR):
                        psq = ps_proj.tile([128, MBS], F32, tag="proj")
                        for t in range(8):
                            nc.tensor.matmul(psq[:], wq_sb[:, t, 128 * p:128 * p + 128], xq[:, t, :],
                                             start=(t == 0), stop=(t == 7))
                        if with_biases:
                            nc.vector.tensor_scalar_add(qT_sb[p][:, m0:m0 + MBS], psq[:], bq_sb[:, p:p + 1])
                        else:
                            nc.vector.tensor_copy(qT_sb[p][:, m0:m0 + MBS], psq[:])
                    return
                xq = xin.tile([128, 8, MBS], BF, tag="xq", name="xq") if parts == 'all' else None
                xk = xin.tile([128, 8, MBS], BF, tag="xk")
                xv = xin.tile([128, 8, MBS], BF, tag="xv")
                if split_in:
                    # interleave weight/input halves so each proj chain's first
                    # matmuls start as soon as possible
                    for w_t, w_sb, src_t, dst in ((wq, wq_sb, qT, xq),
                                                  (wk, wk_sb, kT, xk),
                                                  (wv, wv_sb, vT, xv)):
                        for hf in range(2):
                            nc.sync.dma_start(out=w_sb[:, 4 * hf:4 * hf + 4, :],
                                              in_=w_t[512 * hf:512 * hf + 512, :].rearrange("(t p) n -> p t n", p=128))
                            nc.sync.dma_start(
                                out=dst[:, 4 * hf:4 * hf + 4, :],
                                in_=src_t[512 * hf:512 * hf + 512, m0:m0 + MBS].rearrange(
                                    "(t p) m -> p t m", p=128))
                else:
                    if parts == 'all':
                        nc.sync.dma_start(out=xq, in_=qT[:, m0:m0 + MBS].rearrange("(t p) m -> p t m", p=128))
                    nc.sync.dma_start(out=xk, in_=kT[:, m0:m0 + MBS].rearrange("(t p) m -> p t m", p=128))
                    nc.sync.dma_start(out=xv, in_=vT[:, m0:m0 + MBS].rearrange("(t p) m -> p t m", p=128))
                # For the LAST m-block the Q/K PSUM->SBUF copies run on ACT
                # (Copy is in every function table, so no table swap): at the
                # attn2->attn3 boundary ACT is otherwise idle waiting for
                # scores while DVE is congested with attn2's normalize chain;
                # these copies are exactly what gates attn3's first scores.
                def qk_copy(dst_ap, src_ps, bias_ap):
                    if qk_copies_on_act:
                        nc.scalar.activation(out=dst_ap, in_=src_ps,
                                             func=mybir.ActivationFunctionType.Copy,
                                             bias=bias_ap if with_biases else 0.0,
                                             scale=1.0)
                    elif with_biases:
                        nc.vector.tensor_scalar_add(dst_ap, src_ps, bias_ap)
                    else:
                        nc.vector.tensor_copy(dst_ap, src_ps)
                for p in range(NPAIR):
                    if parts == 'all':
                        psq = ps_proj.tile([128, MBS], F32, tag="proj")
                        for t in range(8):
                            nc.tensor.matmul(psq[:], wq_sb[:, t, 128 * p:128 * p + 128], xq[:, t, :],
                                             start=(t == 0), stop=(t == 7))
                        qk_copy(qT_sb[p][:, m0:m0 + MBS], psq[:], bq_sb[:, p:p + 1])
                    psk = ps_proj.tile([128, MBS], F32, tag="proj")
                    for t in range(8):
                        nc.tensor.matmul(psk[:], wk_sb[:, t, 128 * p:128 * p + 128], xk[:, t, :],
                                         start=(t == 0), stop=(t == 7))
                    qk_copy(kT_sb[p][:, m0:m0 + MBS], psk[:], bk_sb[:, p:p + 1])
                for ms in range(MBS // 128):
                    mt = mb * (MBS // 128) + ms
                    psv = ps_proj.tile([128, HL * DH], F32, tag="proj")
                    for t in range(8):
                        nc.tensor.matmul(psv[:], xv[:, t, 128 * ms:128 * ms + 128], wv_sb[:, t, :],
                                         start=(t == 0), stop=(t == 7))
                    # write into v65 slots (strided dest); bv is added post-normalize
                    dst = v65_sb[:, mt, :].rearrange("p (h x) -> p h x", x=65)[:, :, 0:DH]
                    nc.vector.tensor_copy(dst, psv[:].rearrange("p (h x) -> p h x", x=DH))

            def attn_block(qb):
                q0 = qb * QBS
                nkt = 4 * qb + 4
                for p in range(NPAIR):
                    at_e = ps_acc.tile([65, QBS], F32, tag="acc", name="at_e")
                    at_o = ps_acc.tile([65, QBS], F32, tag="acc", name="at_o")
                    es_prev = None
                    for kt in range(nkt):
                        k0 = kt * KTS
                        # causally-valid q-slice of this tile (cols < off are fully masked)
                        d = kt - 4 * qb
                        off = 128 * d if d > 0 else 0
                        s_e = ps_s.tile([128, QBS], F32, tag="s")
                        s_o = ps_s.tile([128, QBS], F32, tag="s")
                        nc.tensor.matmul(s_e[:, off:], kT_sb[p][0:64, k0:k0 + KTS],
                                         qT_sb[p][0:64, q0 + off:q0 + QBS], start=True, stop=True)
                        nc.tensor.matmul(s_o[:, off:], kT_sb[p][64:128, k0:k0 + KTS],
                                         qT_sb[p][64:128, q0 + off:q0 + QBS], start=True, stop=True)
                        es_e = espool.tile([128, QBS], BF, tag="es_e")
                        es_o = espool.tile([128, QBS], BF, tag="es_o")
                        nc.scalar.activation(out=es_e[:, off:], in_=s_e[:, off:],
                                             func=mybir.ActivationFunctionType.Exp, scale=SCALE)
                        nc.scalar.activation(out=es_o[:, off:], in_=s_o[:, off:],
                                             func=mybir.ActivationFunctionType.Exp, scale=SCALE)
                        if d >= 0:  # diagonal-crossing tile: causal mask
                            for es in (es_e, es_o):
                                nc.vector.tensor_mul(es[:, off:], es[:, off:],
                                                     cmask_sb[:, d, off:])
                        # PV for previous kt was already emitted; emit this kt's PV now.
                        # (software pipeline: scores of kt+1 queue ahead of PV of kt on PE)
                        if es_prev is not None:
                            pkt, poff, pes_e, pes_o = es_prev
                            nc.tensor.matmul(at_e[:, poff:], v65_sb[:, pkt, 65 * 2 * p:65 * 2 * p + 65],
                                             pes_e[:, poff:], start=(pkt == 0), stop=False)
                            nc.tensor.matmul(at_o[:, poff:], v65_sb[:, pkt, 65 * (2 * p + 1):65 * (2 * p + 1) + 65],
                                             pes_o[:, poff:], start=(pkt == 0), stop=False)
                        es_prev = (kt, off, es_e, es_o)
                    pkt, poff, pes_e, pes_o = es_prev
                    nc.tensor.matmul(at_e[:, poff:], v65_sb[:, pkt, 65 * 2 * p:65 * 2 * p + 65],
                                     pes_e[:, poff:], start=(pkt == 0), stop=True)
                    nc.tensor.matmul(at_o[:, poff:], v65_sb[:, pkt, 65 * (2 * p + 1):65 * (2 * p + 1) + 65],
                                     pes_o[:, poff:], start=(pkt == 0), stop=True)
                    # normalize: A = A_unnorm * (1/colsum) broadcast via PE
                    for par, at in ((0, at_e), (1, at_o)):
                        h = 2 * p + par
                        rec = work.tile([65, QBS], BF, tag="rec")
                        with nc.allow_low_precision("bf16 softmax reciprocal is within tolerance"):
                            nc.vector.reciprocal(out=rec[64:65, :], in_=at[64:65, :])
                        bc_ps = ps_bc.tile([64, QBS], F32, tag="bc")
                        bc = bc_ps[:]
                        nc.tensor.matmul(bc, ones_sb[64:65, 0:DH], rec[64:65, :],
                                         start=True, stop=True)
                        bc_sb = work.tile([64, QBS], BF, tag="bc_sb")
                        nc.vector.tensor_copy(bc_sb[:], bc)
                        nc.vector.tensor_mul(a4_sb[h][:, q0:q0 + QBS], at[0:64, :], bc_sb[:])
                        if with_biases:
                            nc.vector.tensor_scalar_add(a4_sb[h][:, q0:q0 + QBS],
                                                        a4_sb[h][:, q0:q0 + QBS],
                                                        bv_sb[:, h:h + 1])
                # A2A input for this q-block: one DMA per head (fires as soon
                # as that head's normalize lands). Row in the group tile:
                # j*256k + qi*256 + 64h + p  (r = 4qi+h below).
                gi, qi = qb2group[qb]
                k = len(A2A_GROUPS[gi])
                for h in range(HL):
                    nc.sync.dma_start(
                        out=in_g[gi].rearrange("(j r p) m -> p r j m",
                                               p=DH, r=4 * k)[:, 4 * qi + h],
                        in_=a4_sb[h][:, q0:q0 + QBS].rearrange("p (j m) -> p j m", m=DH))
                if qb == A2A_GROUPS[gi][-1]:
                    nc.gpsimd.collective_compute(
                        "AllToAll", mybir.AluOpType.bypass,
                        ins=[in_g[gi].opt()], outs=[out_g[gi].opt()],
                        replica_groups=[list(range(NCORES))])

            def gather_block(qb):
                gi, qi = qb2group[qb]
                k = len(A2A_GROUPS[gi])
                if k == 1:
                    # contract chunk t_full = 8x + ct; one DMA per batch-half
                    for x in range(2):
                        nc.sync.dma_start(
                            out=ob_sb[qb][:, :, x, :],
                            in_=out_g[gi][1024 * x:1024 * x + 1024, :].rearrange(
                                "(t p) m -> p t m", p=128))
                    return
                # grouped exchange: qb's rows are strided through the peer
                # blocks; one DMA per (batch-half x, head h) keeps every AP
                # within the 3-dim DMA limit
                src_v = out_g[gi].rearrange("(x g r p) m -> p x g r m",
                                            x=2, g=4, r=4 * k)
                for x in range(2):
                    for h in range(HL):
                        hb = 1 if h >= 2 else 0
                        nc.sync.dma_start(
                            out=ob_sb[qb][64 * (h % 2):64 * (h % 2) + 64,
                                          hb::2, x, :],
                            in_=src_v[:, x, :, 4 * qi + h, :])

            def keep_warm(n):
                """Dummy matmuls that keep the PE clock ramped through the
                final collective's window, so the last out-projection doesn't
                run at the cold p-state (HAM re-throttles after ~3.4us idle).
                Output goes to the bc PSUM bank (idle by then); results are
                never read."""
                for _ in range(n):
                    warm_ps = ps_bc.tile([64, QBS], F32, tag="bc")
                    nc.tensor.matmul(warm_ps[:], ones_sb[64:65, 0:DH],
                                     qT_sb[0][64:65, 0:QBS], start=True, stop=True)

            x_sbs = {}
            mv_sbs = {}

            def epilogue_front(qb, warm=0):
                """out-proj + residual + LN stats for q-block qb.

                warm > 0: first emit `warm` dummy matmuls into the same PSUM
                tiles the real chains will use (WAW pins them BEFORE the real
                matmuls in the PE stream) so the PE clock stays ramped through
                the final collective's window instead of going cold (HAM
                re-throttles after ~3.4us idle). The real chains start with
                start=True, which discards the dummy results."""
                x_sb = epi.tile([128, D], F32, tag=f"x{qb}")
                x_sbs[qb] = x_sb
                qres_sb = qres_sb4[:, qb, :]
                # out-proj PSUM comes from the ATTENTION accumulator pool,
                # not ps_proj: the next rep's first projection chains rotate
                # through ps_proj, and sharing it with the epilogue would make
                # them wait on this rep's final x-adds (serializing reps).
                o_pss = [ps_acc.tile([128, 512], F32, tag="acc",
                                     name=f"o_ps{qb}_{nb}")
                         for nb in range(2)]
                if warm:
                    # bulk dummies rotate through the bc bank: its WAR chain
                    # pins them after the last attention normalize, so they
                    # can't displace the normalize -> in_bq -> collective path
                    for i in range(warm - 2):
                        wp = ps_bc.tile([64, QBS], F32, tag="bc", name=f"warm{i}")
                        nc.tensor.matmul(wp[:], ones_sb[64:65, 0:DH],
                                         qT_sb[0][64:65, 0:QBS], start=True, stop=True)
                    # last two land in the real out-proj tiles: the real
                    # chains' WAW then provably follows every dummy
                    for j in range(2):
                        nc.tensor.matmul(o_pss[j][0:64, :], ones_sb[0:1, 0:DH],
                                         qT_sb[0][0:1, 0:QBS], start=True, stop=True)
                for nb in range(2):
                    o_ps = o_pss[nb]
                    for ct in range(8):
                        nc.tensor.matmul(o_ps[:],
                                         ob_v[qb][:, ct, :, :],
                                         wo_sb[:, ct, 512 * nb:512 * nb + 512],
                                         start=(ct == 0), stop=(ct == 7))
                    nc.vector.tensor_add(x_sb[:, 512 * nb:512 * nb + 512], o_ps[:],
                                         qres_sb[:, 512 * nb:512 * nb + 512])
                stats = work.tile([128, 2, 6], F32, tag="stats")
                nc.vector.bn_stats(out=stats[:, 0, :], in_=x_sb[:, 0:512])
                nc.vector.bn_stats(out=stats[:, 1, :], in_=x_sb[:, 512:1024])
                mv = epi.tile([128, 2], F32, tag=f"mv{qb}")
                mv_sbs[qb] = mv
                nc.vector.bn_aggr(out=mv[:], in_=stats[:])

            def epilogue_back(qb):
                """rstd (Newton rsqrt on gpsimd: no ACT table swap away from
                Exp, and parks only the idle Pool queue) + normalize + store."""
                x_sb, mv = x_sbs[qb], mv_sbs[qb]
                vh = epi.tile([128, 1], F32, tag=f"vh{qb}")
                nc.gpsimd.tensor_scalar(out=vh[:], in0=mv[:, 1:2],
                                        scalar1=0.5, scalar2=0.5 * LN_EPS,
                                        op0=mybir.AluOpType.mult,
                                        op1=mybir.AluOpType.add)
                den = epi.tile([128, 1], F32, tag=f"den{qb}")
                nc.gpsimd.tensor_scalar_add(den[:], vh[:], 0.5)
                rstd = epi.tile([128, 1], F32, tag=f"rstd{qb}")
                nc.vector.reciprocal(out=rstd[:], in_=den[:])
                for _it in range(3):
                    rr = epi.tile([128, 1], F32, tag=f"rr{qb}")
                    nc.gpsimd.tensor_mul(rr[:], rstd[:], rstd[:])
                    nc.gpsimd.tensor_mul(rr[:], rr[:], vh[:])
                    nc.gpsimd.tensor_scalar(out=rr[:], in0=rr[:],
                                            scalar1=-1.0, scalar2=1.5,
                                            op0=mybir.AluOpType.mult,
                                            op1=mybir.AluOpType.add)
                    nc.gpsimd.tensor_mul(rstd[:], rstd[:], rr[:])
                y_sb = work.tile([128, D], F32, tag="y")
                nc.vector.tensor_scalar(out=y_sb[:], in0=x_sb[:],
                                        scalar1=mv[:, 0:1], scalar2=rstd[:, 0:1],
                                        op0=mybir.AluOpType.subtract,
                                        op1=mybir.AluOpType.mult)
                if ln_affine:
                    nc.gpsimd.tensor_mul(y_sb[:], y_sb[:], gam_sb[:])
                    nc.gpsimd.tensor_add(y_sb[:], y_sb[:], bet_sb[:])
                nc.sync.dma_start(out=y[128 * qb:128 * qb + 128, :], in_=y_sb[:])

            for _rep in range(reps):
              # Emission order: gather(qb) right after attn(qb) so its wait
              # on the shared Collectives semaphore stays precise (>= qb+1);
              # epilogues 0/1 before the last attention block so their work
              # fills its ACT-bound bubbles; epilogue 2 + keep-warm matmuls
              # cover the final A2A window so the last out-proj runs warm.
              for i in range(NMB):
                proj_block(i, split_in=(_rep == 0 and i == 0))
                if _rep == 0 and i == 1:
                    # E-phase consts: load while attention runs, off the startup path
                    for hf in range(2):
                        nc.sync.dma_start(out=wo_sb[:, :, 512 * hf:512 * hf + 512],
                                          in_=wo[:, 512 * hf:512 * hf + 512].rearrange(
                                              "(t p) n -> p t n", p=128))
                    nc.sync.dma_start(out=qres_sb4, in_=qresbo.rearrange("(r p) n -> p r n", p=128))
                    if ln_affine:
                        nc.sync.dma_start(out=gam_sb, in_=gamma[:, :].to_broadcast([128, D]))
                        nc.sync.dma_start(out=bet_sb, in_=beta[:, :].to_broadcast([128, D]))
                if phases != 'proj':
                    attn_block(i)
                    if phases == 'full' and any(i == g[-1] for g in A2A_GROUPS):
                        # gathers for every member, right after this group's
                        # collective and before the next one (precise waits)
                        for qb in next(g for g in A2A_GROUPS if g[-1] == i):
                            gather_block(qb)
              if phases == 'full':
                  for qb in range(NQB):
                      epilogue_front(qb)
                      epilogue_back(qb)
    nc.finalize()
    return nc


_CACHE = {}


def _prep_inputs(query, key, value, Wq, bq, Wk, bk, Wv, bv, Wo, bo, gamma, beta):
    """Host-side shard + transpose + cast. Returns per-core in_maps."""
    q32 = np.asarray(query, np.float32)
    qT = [np.ascontiguousarray(q32[b].T).astype(BF16) for b in range(B)]
    kTt = [np.ascontiguousarray(np.asarray(key, np.float32)[b].T).astype(BF16) for b in range(B)]
    vTt = [np.ascontiguousarray(np.asarray(value, np.float32)[b].T).astype(BF16) for b in range(B)]
    Wqb = np.asarray(Wq, np.float32).astype(BF16)
    Wkb = np.asarray(Wk, np.float32).astype(BF16)
    Wvb = np.asarray(Wv, np.float32).astype(BF16)
    Wob = np.ascontiguousarray(np.asarray(Wo, np.float32)).astype(BF16)
    bo32 = np.asarray(bo, np.float32)
    in_maps = []
    for c in range(NCORES):
        b, g = divmod(c, 4)
        sl = slice(HL * DH * g, HL * DH * (g + 1))
        # rows ordered (qb, batch, i)
        qres = np.concatenate(
            [q32[b_, 512 * qb + 64 * c: 512 * qb + 64 * c + 64] + bo32
             for qb in range(4) for b_ in range(B)], axis=0)
        in_maps.append({
            "qT": qT[b], "kT": kTt[b], "vT": vTt[b],
            "wq": np.ascontiguousarray(Wqb[:, sl]),
            "wk": np.ascontiguousarray(Wkb[:, sl]),
            "wv": np.ascontiguousarray(Wvb[:, sl]),
            "wo": Wob,
            "bq": np.ascontiguousarray(np.asarray(bq, np.float32)[sl]).reshape(HL * DH, 1),
            "bk": np.ascontiguousarray(np.asarray(bk, np.float32)[sl]).reshape(HL * DH, 1),
            "bv": np.ascontiguousarray(np.asarray(bv, np.float32)[sl].reshape(HL, DH).T),
            "qresbo": np.ascontiguousarray(qres, np.float32),
            "gamma": np.asarray(gamma, np.float32).reshape(1, D),
            "beta": np.asarray(beta, np.float32).reshape(1, D),
        })
    return in_maps


def _assemble(results):
    out = np.empty((B, L, D), np.float32)
    for c in range(NCORES):
        yc = results[c]["y"]
        for qb in range(4):
            for b_ in range(B):
                out[b_, 512 * qb + 64 * c: 512 * qb + 64 * c + 64] = \
                    yc[128 * qb + 64 * b_: 128 * qb + 64 * b_ + 64]
    return out


def kernel(**inputs) -> np.ndarray:
    from concourse.bass_utils import run_bass_kernel_spmd
    in_maps = _prep_inputs(
        inputs["query"], inputs["key"], inputs["value"],
        inputs["Wq"], inputs["bq"], inputs["Wk"], inputs["bk"],
        inputs["Wv"], inputs["bv"], inputs["Wo"], inputs["bo"],
        inputs["gamma"], inputs["beta"])
    wb = any(np.any(np.asarray(inputs[k]) != 0) for k in ("bq", "bk", "bv"))
    la = (np.any(np.asarray(inputs["gamma"]) != 1.0)
          or np.any(np.asarray(inputs["beta"]) != 0.0))
    key = ("nc", wb, la)
    if key not in _CACHE:
        _CACHE[key] = build_nc(with_biases=wb, ln_affine=la)
    _CACHE["nc"] = _CACHE[key]
    res = run_bass_kernel_spmd(_CACHE[key], in_maps, core_ids=list(range(NCORES)))
    return _assemble(res.results)


if __name__ == "__main__":
    # quick shape check of the program build
    nc = build_nc()
    n_inst = sum(len(bb.instructions) for f in nc.m.functions for bb in f.blocks)
    print("built ok, instructions:", n_inst)


# revision 89
# speedup vs baseline: 1.6549x; 1.6549x over previous
"""Trainium2 Bass kernel for causal MultiHeadAttention + residual + LayerNorm.

Problem: nn_MultiHeadAttention_88124138979383
  B=2, L=2048, D=1024, H=16, DH=64, causal mask, out-proj, residual, LN.

Sharding (8 cores): core c = (batch b=c//4, head-group g=c%4, 4 heads each).
Each core projects Q^T/K^T (head-dim on partitions) and V (natural layout)
for its batch+heads, runs causal attention in scores-transposed layout
(softmax reduction via a ones-column appended to V inside the P@V matmul,
no max-subtraction — scores are small), normalizes via a PE-broadcast of
the reciprocal denominator, then per q-block an 8-core AllToAll exchanges
head-shards for sequence-shards: after A2A #qb, core c has the full
16-head A^T for rows [512qb+64c, +64) of BOTH batches; it immediately runs
the output projection (both batches packed on 128 partitions), residual
(query rows + bo pre-added on host) and LayerNorm for those rows, so the
epilogue of q-block qb overlaps attention of q-block qb+1.

All matmuls in bf16 (fp32 PSUM accumulate). Host transposes/casts inputs.
"""
import os
import sys

for _p in ("/opt/trn_rl_repo", os.path.join(os.path.expanduser("~"), ".axon_site", "_ro", "trn_rl_repo")):
    if os.path.isdir(_p) and _p not in sys.path:
        sys.path.insert(0, _p)

import numpy as np
import ml_dtypes

import concourse.bass as bass
import concourse.tile as tile
from concourse import bacc, mybir

BF16 = ml_dtypes.bfloat16
F32 = mybir.dt.float32
BF = mybir.dt.bfloat16

B, L, D = 2, 2048, 1024
H, DH = 16, 64
NCORES = 8
HL = 4                 # heads per core
NPAIR = 2              # head pairs per core
MBS = 512              # m-block size for projections
NMB = L // MBS         # 4
QBS = 512              # q-block size for attention
NQB = L // QBS         # 4
KTS = 128              # k-tile size
NKT = L // KTS         # 16
MS = L // NCORES       # 256: per-core row-slice per A2A block
LN_EPS = 1e-5
SCALE = 1.0 / 8.0      # 1/sqrt(DH)


def build_nc(reps: int = 1, phases: str = 'full', with_biases: bool = False,
             ln_affine: bool = False):
    nc = bacc.Bacc("TRN2", target_bir_lowering=False, debug=False, num_devices=NCORES)
    qT = nc.dram_tensor("qT", [D, L], BF, kind="ExternalInput")
    kT = nc.dram_tensor("kT", [D, L], BF, kind="ExternalInput")
    vT = nc.dram_tensor("vT", [D, L], BF, kind="ExternalInput")
    wq = nc.dram_tensor("wq", [D, HL * DH], BF, kind="ExternalInput")
    wk = nc.dram_tensor("wk", [D, HL * DH], BF, kind="ExternalInput")
    wv = nc.dram_tensor("wv", [D, HL * DH], BF, kind="ExternalInput")
    wo = nc.dram_tensor("wo", [D, D], BF, kind="ExternalInput")
    bq = nc.dram_tensor("bq", [HL * DH, 1], F32, kind="ExternalInput")
    bk = nc.dram_tensor("bk", [HL * DH, 1], F32, kind="ExternalInput")
    bv = nc.dram_tensor("bv", [DH, HL], F32, kind="ExternalInput")
    # rows ordered (qb, batch, i): row 128*qb + 64*b + i = query[b, 512qb+64c+i] + bo
    qresbo = nc.dram_tensor("qresbo", [4 * 128, D], F32, kind="ExternalInput")
    gamma = nc.dram_tensor("gamma", [1, D], F32, kind="ExternalInput")
    beta = nc.dram_tensor("beta", [1, D], F32, kind="ExternalInput")
    y = nc.dram_tensor("y", [4 * 128, D], F32, kind="ExternalOutput")

    with tile.TileContext(nc) as tc:
        with (
            tc.tile_pool(name="consts", bufs=1) as consts,
            tc.tile_pool(name="persist", bufs=1) as persist,
            tc.tile_pool(name="xin", bufs=2) as xin,
            tc.tile_pool(name="es", bufs=8) as espool,
            tc.tile_pool(name="work", bufs=2) as work,
            tc.tile_pool(name="epi", bufs=1) as epi,
            tc.tile_pool(name="ps_proj", bufs=2, space="PSUM") as ps_proj,
            tc.tile_pool(name="ps_s", bufs=3, space="PSUM") as ps_s,
            tc.tile_pool(name="ps_acc", bufs=2, space="PSUM") as ps_acc,
            tc.tile_pool(name="ps_bc", bufs=1, space="PSUM") as ps_bc,
            tc.tile_pool(name="dram", bufs=1, space="DRAM") as dram,
        ):
            # ---- constants / weights ----
            wq_sb = consts.tile([128, 8, HL * DH], BF, tag="wq")
            wk_sb = consts.tile([128, 8, HL * DH], BF, tag="wk")
            wv_sb = consts.tile([128, 8, HL * DH], BF, tag="wv")

            def load_wkv():
                nc.sync.dma_start(out=wk_sb, in_=wk.rearrange("(t p) n -> p t n", p=128))
                nc.sync.dma_start(out=wv_sb, in_=wv.rearrange("(t p) n -> p t n", p=128))
            wo_sb = consts.tile([128, 8, D], BF, tag="wo")
            bq_sb = consts.tile([128, NPAIR], F32, tag="bq")
            bk_sb = consts.tile([128, NPAIR], F32, tag="bk")
            if with_biases:
                nc.sync.dma_start(out=bq_sb, in_=bq.rearrange("(t p) o -> p (t o)", p=128))
                nc.sync.dma_start(out=bk_sb, in_=bk.rearrange("(t p) o -> p (t o)", p=128))
            bv_sb = consts.tile([DH, HL], F32, tag="bv")
            if with_biases:
                nc.sync.dma_start(out=bv_sb, in_=bv[:, :])
            ones_sb = consts.tile([128, DH], BF, tag="ones")
            nc.gpsimd.memset(ones_sb, 1.0)
            # causal masks for the 4 diagonal-band offsets d = kt - 4*qb:
            # mask_d[p, q] = 1.0 where q >= p + 128*d else 0.0
            cmask_sb = consts.tile([128, 4, QBS], BF, tag="cmask")
            nc.gpsimd.memset(cmask_sb, 1.0)
            for d in range(4):
                nc.gpsimd.affine_select(
                    out=cmask_sb[:, d, :], in_=cmask_sb[:, d, :],
                    compare_op=mybir.AluOpType.is_ge, fill=0.0,
                    base=-128 * d, channel_multiplier=-1, pattern=[[1, QBS]])

            # ---- persistent activations ----
            qT_sb = [persist.tile([128, L], BF, tag=f"qT{p}", name=f"qT_sb{p}") for p in range(NPAIR)]
            kT_sb = [persist.tile([128, L], BF, tag=f"kT{p}", name=f"kT_sb{p}") for p in range(NPAIR)]
            # V in natural [seq, d] layout, 65 cols per head (col 64 = ones)
            v65_sb = persist.tile([128, NKT, HL * 65], BF, tag="v65")
            nc.gpsimd.memset(v65_sb, 1.0)
            # normalized attention output A^T, one tile per head so readers
            # only wait on their own head's writes: [DH, L]
            a4_sb = [persist.tile([DH, L], BF, tag=f"a4_{h}", name=f"a4_sb{h}")
                     for h in range(HL)]
            # gathered A^T after chunked A2As, one tile per q-block so the
            # epilogue of qb only waits on its own gather DMAs. Layout
            # [p, ct, x(batch), m] keeps the out-proj lhsT slice (x, m)
            # contiguous -- walrus requires a single mergeable free dim.
            ob_sb = [persist.tile([128, 8, 2, DH], BF, tag=f"ob{q}", name=f"ob_sb{q}")
                     for q in range(NQB)]
            ob_v = ob_sb

            gam_sb = consts.tile([128, D], F32, tag="gam") if ln_affine else None
            bet_sb = consts.tile([128, D], F32, tag="bet") if ln_affine else None
            qres_sb4 = persist.tile([128, 4, D], F32, tag="qres4", name="qres_sb4")

            in_bq = [dram.tile([L, DH], BF, name=f"in_bq{i}") for i in range(NQB)]
            out_bq = [dram.tile([L, DH], BF, name=f"out_bq{i}") for i in range(NQB)]

            def proj_block(mb, split_in=False, qk_copies_on_act=False, parts='all'):
                m0 = mb * MBS
                if parts == 'q':
                    # Q-only part, hoisted one attention block early: the exp
                    # stream of block mb is gated only by qT(mb) (its K/V are
                    # needed just for the last 4 diagonal k-tiles), so copying
                    # Q during attn(mb-1) lets ACT start block mb's exps
                    # without a boundary gap.
                    xq = xin.tile([128, 8, MBS], BF, tag="xq")
                    nc.sync.dma_start(out=xq, in_=qT[:, m0:m0 + MBS].rearrange("(t p) m -> p t m", p=128))
                    for p in range(NPAIR):
                        psq = ps_proj.tile([128, MBS], F32, tag="proj")
                        for t in range(8):
                            nc.tensor.matmul(psq[:], wq_sb[:, t, 128 * p:128 * p + 128], xq[:, t, :],
                                             start=(t == 0), stop=(t == 7))
                        if with_biases:
                            nc.vector.tensor_scalar_add(qT_sb[p][:, m0:m0 + MBS], psq[:], bq_sb[:, p:p + 1])
                        else:
                            nc.vector.tensor_copy(qT_sb[p][:, m0:m0 + MBS], psq[:])
                    return
                xq = xin.tile([128, 8, MBS], BF, tag="xq", name="xq") if parts == 'all' else None
                xk = xin.tile([128, 8, MBS], BF, tag="xk")
                xv = xin.tile([128, 8, MBS], BF, tag="xv")
                if split_in:
                    # interleave weight/input halves so each proj chain's first
                    # matmuls start as soon as possible
                    for w_t, w_sb, src_t, dst in ((wq, wq_sb, qT, xq),
                                                  (wk, wk_sb, kT, xk),
                                                  (wv, wv_sb, vT, xv)):
                        for hf in range(2):
                            nc.sync.dma_start(out=w_sb[:, 4 * hf:4 * hf + 4, :],
                                              in_=w_t[512 * hf:512 * hf + 512, :].rearrange("(t p) n -> p t n", p=128))
                            nc.sync.dma_start(
                                out=dst[:, 4 * hf:4 * hf + 4, :],
                                in_=src_t[512 * hf:512 * hf + 512, m0:m0 + MBS].rearrange(
                                    "(t p) m -> p t m", p=128))
                else:
                    if parts == 'all':
                        nc.sync.dma_start(out=xq, in_=qT[:, m0:m0 + MBS].rearrange("(t p) m -> p t m", p=128))
                    nc.sync.dma_start(out=xk, in_=kT[:, m0:m0 + MBS].rearrange("(t p) m -> p t m", p=128))
                    nc.sync.dma_start(out=xv, in_=vT[:, m0:m0 + MBS].rearrange("(t p) m -> p t m", p=128))
                # For the LAST m-block the Q/K PSUM->SBUF copies run on ACT
                # (Copy is in every function table, so no table swap): at the
                # attn2->attn3 boundary ACT is otherwise idle waiting for
                # scores while DVE is congested with attn2's normalize chain;
                # these copies are exactly what gates attn3's first scores.
                def qk_copy(dst_ap, src_ps, bias_ap):
                    if qk_copies_on_act:
                        nc.scalar.activation(out=dst_ap, in_=src_ps,
                                             func=mybir.ActivationFunctionType.Copy,
                                             bias=bias_ap if with_biases else 0.0,
                                             scale=1.0)
                    elif with_biases:
                        nc.vector.tensor_scalar_add(dst_ap, src_ps, bias_ap)
                    else:
                        nc.vector.tensor_copy(dst_ap, src_ps)
                for p in range(NPAIR):
                    if parts == 'all':
                        psq = ps_proj.tile([128, MBS], F32, tag="proj")
                        for t in range(8):
                            nc.tensor.matmul(psq[:], wq_sb[:, t, 128 * p:128 * p + 128], xq[:, t, :],
                                             start=(t == 0), stop=(t == 7))
                        qk_copy(qT_sb[p][:, m0:m0 + MBS], psq[:], bq_sb[:, p:p + 1])
                    psk = ps_proj.tile([128, MBS], F32, tag="proj")
                    for t in range(8):
                        nc.tensor.matmul(psk[:], wk_sb[:, t, 128 * p:128 * p + 128], xk[:, t, :],
                                         start=(t == 0), stop=(t == 7))
                    qk_copy(kT_sb[p][:, m0:m0 + MBS], psk[:], bk_sb[:, p:p + 1])
                for ms in range(MBS // 128):
                    mt = mb * (MBS // 128) + ms
                    psv = ps_proj.tile([128, HL * DH], F32, tag="proj")
                    for t in range(8):
                        nc.tensor.matmul(psv[:], xv[:, t, 128 * ms:128 * ms + 128], wv_sb[:, t, :],
                                         start=(t == 0), stop=(t == 7))
                    # write into v65 slots (strided dest); bv is added post-normalize
                    dst = v65_sb[:, mt, :].rearrange("p (h x) -> p h x", x=65)[:, :, 0:DH]
                    nc.vector.tensor_copy(dst, psv[:].rearrange("p (h x) -> p h x", x=DH))

            def attn_block(qb):
                q0 = qb * QBS
                nkt = 4 * qb + 4
                for p in range(NPAIR):
                    at_e = ps_acc.tile([65, QBS], F32, tag="acc", name="at_e")
                    at_o = ps_acc.tile([65, QBS], F32, tag="acc", name="at_o")
                    es_prev = None
                    for kt in range(nkt):
                        k0 = kt * KTS
                        # causally-valid q-slice of this tile (cols < off are fully masked)
                        d = kt - 4 * qb
                        off = 128 * d if d > 0 else 0
                        s_e = ps_s.tile([128, QBS], F32, tag="s")
                        s_o = ps_s.tile([128, QBS], F32, tag="s")
                        nc.tensor.matmul(s_e[:, off:], kT_sb[p][0:64, k0:k0 + KTS],
                                         qT_sb[p][0:64, q0 + off:q0 + QBS], start=True, stop=True)
                        nc.tensor.matmul(s_o[:, off:], kT_sb[p][64:128, k0:k0 + KTS],
                                         qT_sb[p][64:128, q0 + off:q0 + QBS], start=True, stop=True)
                        es_e = espool.tile([128, QBS], BF, tag="es_e")
                        es_o = espool.tile([128, QBS], BF, tag="es_o")
                        nc.scalar.activation(out=es_e[:, off:], in_=s_e[:, off:],
                                             func=mybir.ActivationFunctionType.Exp, scale=SCALE)
                        nc.scalar.activation(out=es_o[:, off:], in_=s_o[:, off:],
                                             func=mybir.ActivationFunctionType.Exp, scale=SCALE)
                        if d >= 0:  # diagonal-crossing tile: causal mask
                            for es in (es_e, es_o):
                                nc.vector.tensor_mul(es[:, off:], es[:, off:],
                                                     cmask_sb[:, d, off:])
                        # PV for previous kt was already emitted; emit this kt's PV now.
                        # (software pipeline: scores of kt+1 queue ahead of PV of kt on PE)
                        if es_prev is not None:
                            pkt, poff, pes_e, pes_o = es_prev
                            nc.tensor.matmul(at_e[:, poff:], v65_sb[:, pkt, 65 * 2 * p:65 * 2 * p + 65],
                                             pes_e[:, poff:], start=(pkt == 0), stop=False)
                            nc.tensor.matmul(at_o[:, poff:], v65_sb[:, pkt, 65 * (2 * p + 1):65 * (2 * p + 1) + 65],
                                             pes_o[:, poff:], start=(pkt == 0), stop=False)
                        es_prev = (kt, off, es_e, es_o)
                    pkt, poff, pes_e, pes_o = es_prev
                    nc.tensor.matmul(at_e[:, poff:], v65_sb[:, pkt, 65 * 2 * p:65 * 2 * p + 65],
                                     pes_e[:, poff:], start=(pkt == 0), stop=True)
                    nc.tensor.matmul(at_o[:, poff:], v65_sb[:, pkt, 65 * (2 * p + 1):65 * (2 * p + 1) + 65],
                                     pes_o[:, poff:], start=(pkt == 0), stop=True)
                    # normalize: A = A_unnorm * (1/colsum) broadcast via PE
                    for par, at in ((0, at_e), (1, at_o)):
                        h = 2 * p + par
                        rec = work.tile([65, QBS], BF, tag="rec")
                        with nc.allow_low_precision("bf16 softmax reciprocal is within tolerance"):
                            nc.vector.reciprocal(out=rec[64:65, :], in_=at[64:65, :])
                        bc_ps = ps_bc.tile([64, QBS], F32, tag="bc")
                        bc = bc_ps[:]
                        nc.tensor.matmul(bc, ones_sb[64:65, 0:DH], rec[64:65, :],
                                         start=True, stop=True)
                        bc_sb = work.tile([64, QBS], BF, tag="bc_sb")
                        nc.vector.tensor_copy(bc_sb[:], bc)
                        nc.vector.tensor_mul(a4_sb[h][:, q0:q0 + QBS], at[0:64, :], bc_sb[:])
                        if with_biases:
                            nc.vector.tensor_scalar_add(a4_sb[h][:, q0:q0 + QBS],
                                                        a4_sb[h][:, q0:q0 + QBS],
                                                        bv_sb[:, h:h + 1])
                # A2A input for this q-block: one DMA per head (fires as soon as
                # that head's normalize lands); dst rows (j h p), cols m
                for h in range(HL):
                    nc.sync.dma_start(
                        out=in_bq[qb].rearrange("(j h p) m -> p h j m", p=DH, h=HL)[:, h],
                        in_=a4_sb[h][:, q0:q0 + QBS].rearrange("p (j m) -> p j m", m=DH))
                nc.gpsimd.collective_compute(
                    "AllToAll", mybir.AluOpType.bypass,
                    ins=[in_bq[qb].opt()], outs=[out_bq[qb].opt()],
                    replica_groups=[list(range(NCORES))])

            def gather_block(qb):
                # A2A block t_full = 8x + ct; one DMA per batch-half x
                for x in range(2):
                    nc.sync.dma_start(
                        out=ob_sb[qb][:, :, x, :],
                        in_=out_bq[qb][1024 * x:1024 * x + 1024, :].rearrange(
                            "(t p) m -> p t m", p=128))

            def keep_warm(n):
                """Dummy matmuls that keep the PE clock ramped through the
                final collective's window, so the last out-projection doesn't
                run at the cold p-state (HAM re-throttles after ~3.4us idle).
                Output goes to the bc PSUM bank (idle by then); results are
                never read."""
                for _ in range(n):
                    warm_ps = ps_bc.tile([64, QBS], F32, tag="bc")
                    nc.tensor.matmul(warm_ps[:], ones_sb[64:65, 0:DH],
                                     qT_sb[0][64:65, 0:QBS], start=True, stop=True)

            x_sbs = {}
            mv_sbs = {}

            def epilogue_front(qb, warm=0):
                """out-proj + residual + LN stats for q-block qb.

                warm > 0: first emit `warm` dummy matmuls into the same PSUM
                tiles the real chains will use (WAW pins them BEFORE the real
                matmuls in the PE stream) so the PE clock stays ramped through
                the final collective's window instead of going cold (HAM
                re-throttles after ~3.4us idle). The real chains start with
                start=True, which discards the dummy results."""
                x_sb = epi.tile([128, D], F32, tag=f"x{qb}")
                x_sbs[qb] = x_sb
                qres_sb = qres_sb4[:, qb, :]
                # out-proj PSUM comes from the ATTENTION accumulator pool,
                # not ps_proj: the next rep's first projection chains rotate
                # through ps_proj, and sharing it with the epilogue would make
                # them wait on this rep's final x-adds (serializing reps).
                o_pss = [ps_acc.tile([128, 512], F32, tag="acc",
                                     name=f"o_ps{qb}_{nb}")
                         for nb in range(2)]
                if warm:
                    # bulk dummies rotate through the bc bank: its WAR chain
                    # pins them after the last attention normalize, so they
                    # can't displace the normalize -> in_bq -> collective path
                    for i in range(warm - 2):
                        wp = ps_bc.tile([64, QBS], F32, tag="bc", name=f"warm{i}")
                        nc.tensor.matmul(wp[:], ones_sb[64:65, 0:DH],
                                         qT_sb[0][64:65, 0:QBS], start=True, stop=True)
                    # last two land in the real out-proj tiles: the real
                    # chains' WAW then provably follows every dummy
                    for j in range(2):
                        nc.tensor.matmul(o_pss[j][0:64, :], ones_sb[0:1, 0:DH],
                                         qT_sb[0][0:1, 0:QBS], start=True, stop=True)
                for nb in range(2):
                    o_ps = o_pss[nb]
                    for ct in range(8):
                        nc.tensor.matmul(o_ps[:],
                                         ob_v[qb][:, ct, :, :],
                                         wo_sb[:, ct, 512 * nb:512 * nb + 512],
                                         start=(ct == 0), stop=(ct == 7))
                    nc.vector.tensor_add(x_sb[:, 512 * nb:512 * nb + 512], o_ps[:],
                                         qres_sb[:, 512 * nb:512 * nb + 512])
                stats = work.tile([128, 2, 6], F32, tag="stats")
                nc.vector.bn_stats(out=stats[:, 0, :], in_=x_sb[:, 0:512])
                nc.vector.bn_stats(out=stats[:, 1, :], in_=x_sb[:, 512:1024])
                mv = epi.tile([128, 2], F32, tag=f"mv{qb}")
                mv_sbs[qb] = mv
                nc.vector.bn_aggr(out=mv[:], in_=stats[:])

            def epilogue_back(qb):
                """rstd (Newton rsqrt on gpsimd: no ACT table swap away from
                Exp, and parks only the idle Pool queue) + normalize + store."""
                x_sb, mv = x_sbs[qb], mv_sbs[qb]
                vh = epi.tile([128, 1], F32, tag=f"vh{qb}")
                nc.gpsimd.tensor_scalar(out=vh[:], in0=mv[:, 1:2],
                                        scalar1=0.5, scalar2=0.5 * LN_EPS,
                                        op0=mybir.AluOpType.mult,
                                        op1=mybir.AluOpType.add)
                den = epi.tile([128, 1], F32, tag=f"den{qb}")
                nc.gpsimd.tensor_scalar_add(den[:], vh[:], 0.5)
                rstd = epi.tile([128, 1], F32, tag=f"rstd{qb}")
                nc.vector.reciprocal(out=rstd[:], in_=den[:])
                for _it in range(3):
                    rr = epi.tile([128, 1], F32, tag=f"rr{qb}")
                    nc.gpsimd.tensor_mul(rr[:], rstd[:], rstd[:])
                    nc.gpsimd.tensor_mul(rr[:], rr[:], vh[:])
                    nc.gpsimd.tensor_scalar(out=rr[:], in0=rr[:],
                                            scalar1=-1.0, scalar2=1.5,
                                            op0=mybir.AluOpType.mult,
                                            op1=mybir.AluOpType.add)
                    nc.gpsimd.tensor_mul(rstd[:], rstd[:], rr[:])
                y_sb = work.tile([128, D], F32, tag="y")
                nc.vector.tensor_scalar(out=y_sb[:], in0=x_sb[:],
                                        scalar1=mv[:, 0:1], scalar2=rstd[:, 0:1],
                                        op0=mybir.AluOpType.subtract,
                                        op1=mybir.AluOpType.mult)
                if ln_affine:
                    nc.gpsimd.tensor_mul(y_sb[:], y_sb[:], gam_sb[:])
                    nc.gpsimd.tensor_add(y_sb[:], y_sb[:], bet_sb[:])
                nc.sync.dma_start(out=y[128 * qb:128 * qb + 128, :], in_=y_sb[:])

            for _rep in range(reps):
              # Emission order: gather(qb) right after attn(qb) so its wait
              # on the shared Collectives semaphore stays precise (>= qb+1);
              # epilogues 0/1 before the last attention block so their work
              # fills its ACT-bound bubbles; epilogue 2 + keep-warm matmuls
              # cover the final A2A window so the last out-proj runs warm.
              for i in range(NMB):
                proj_block(i, split_in=(_rep == 0 and i == 0))
                if _rep == 0 and i == 1:
                    # E-phase consts: load while attention runs, off the startup path
                    for hf in range(2):
                        nc.sync.dma_start(out=wo_sb[:, :, 512 * hf:512 * hf + 512],
                                          in_=wo[:, 512 * hf:512 * hf + 512].rearrange(
                                              "(t p) n -> p t n", p=128))
                    nc.sync.dma_start(out=qres_sb4, in_=qresbo.rearrange("(r p) n -> p r n", p=128))
                    if ln_affine:
                        nc.sync.dma_start(out=gam_sb, in_=gamma[:, :].to_broadcast([128, D]))
                        nc.sync.dma_start(out=bet_sb, in_=beta[:, :].to_broadcast([128, D]))
                if phases != 'proj':
                    attn_block(i)
                    if phases == 'full':
                        gather_block(i)
              if phases == 'full':
                  for qb in range(NQB):
                      epilogue_front(qb)
                      epilogue_back(qb)
    nc.finalize()
    return nc


_CACHE = {}


def _prep_inputs(query, key, value, Wq, bq, Wk, bk, Wv, bv, Wo, bo, gamma, beta):
    """Host-side shard + transpose + cast. Returns per-core in_maps."""
    q32 = np.asarray(query, np.float32)
    qT = [np.ascontiguousarray(q32[b].T).astype(BF16) for b in range(B)]
    kTt = [np.ascontiguousarray(np.asarray(key, np.float32)[b].T).astype(BF16) for b in range(B)]
    vTt = [np.ascontiguousarray(np.asarray(value, np.float32)[b].T).astype(BF16) for b in range(B)]
    Wqb = np.asarray(Wq, np.float32).astype(BF16)
    Wkb = np.asarray(Wk, np.float32).astype(BF16)
    Wvb = np.asarray(Wv, np.float32).astype(BF16)
    Wob = np.ascontiguousarray(np.asarray(Wo, np.float32)).astype(BF16)
    bo32 = np.asarray(bo, np.float32)
    in_maps = []
    for c in range(NCORES):
        b, g = divmod(c, 4)
        sl = slice(HL * DH * g, HL * DH * (g + 1))
        # rows ordered (qb, batch, i)
        qres = np.concatenate(
            [q32[b_, 512 * qb + 64 * c: 512 * qb + 64 * c + 64] + bo32
             for qb in range(4) for b_ in range(B)], axis=0)
        in_maps.append({
            "qT": qT[b], "kT": kTt[b], "vT": vTt[b],
            "wq": np.ascontiguousarray(Wqb[:, sl]),
            "wk": np.ascontiguousarray(Wkb[:, sl]),
            "wv": np.ascontiguousarray(Wvb[:, sl]),
            "wo": Wob,
            "bq": np.ascontiguousarray(np.asarray(bq, np.float32)[sl]).reshape(HL * DH, 1),
            "bk": np.ascontiguousarray(np.asarray(bk, np.float32)[sl]).reshape(HL * DH, 1),
            "bv": np.ascontiguousarray(np.asarray(bv, np.float32)[sl].reshape(HL, DH).T),
            "qresbo": np.ascontiguousarray(qres, np.float32),
            "gamma": np.asarray(gamma, np.float32).reshape(1, D),
            "beta": np.asarray(beta, np.float32).reshape(1, D),
        })
    return in_maps


def _assemble(results):
    out = np.empty((B, L, D), np.float32)
    for c in range(NCORES):
        yc = results[c]["y"]
        for qb in range(4):
            for b_ in range(B):
                out[b_, 512 * qb + 64 * c: 512 * qb + 64 * c + 64] = \
                    yc[128 * qb + 64 * b_: 128 * qb + 64 * b_ + 64]
    return out


def kernel(**inputs) -> np.ndarray:
    from concourse.bass_utils import run_bass_kernel_spmd
    in_maps = _prep_inputs(
        inputs["query"], inputs["key"], inputs["value"],
        inputs["Wq"], inputs["bq"], inputs["Wk"], inputs["bk"],
        inputs["Wv"], inputs["bv"], inputs["Wo"], inputs["bo"],
        inputs["gamma"], inputs["beta"])
    wb = any(np.any(np.asarray(inputs[k]) != 0) for k in ("bq", "bk", "bv"))
    la = (np.any(np.asarray(inputs["gamma"]) != 1.0)
          or np.any(np.asarray(inputs["beta"]) != 0.0))
    key = ("nc", wb, la)
    if key not in _CACHE:
        _CACHE[key] = build_nc(with_biases=wb, ln_affine=la)
    _CACHE["nc"] = _CACHE[key]
    res = run_bass_kernel_spmd(_CACHE[key], in_maps, core_ids=list(range(NCORES)))
    return _assemble(res.results)


if __name__ == "__main__":
    # quick shape check of the program build
    nc = build_nc()
    n_inst = sum(len(bb.instructions) for f in nc.m.functions for bb in f.blocks)
    print("built ok, instructions:", n_inst)
